# revision 1
# baseline (speedup 1.0000x reference)
"""CrossAttentionNoGate Trainium2 kernel.

Shards the MSA-row dim S (=64) across 8 NeuronCores (8 rows/core, fully
data-parallel, no collectives). Per core, for each s-row:

  qT  [HD,Q]  = Wq^T @ xq^T          (lhsT=Wq chunk, rhs=xqT chunk)
  kT  [HD,KV] = Wk^T @ xkv^T
  v   [KV,HD] = xkv @ Wv             (lhsT=xkvT chunk, rhs=Wv chunk)
  scoresT[kv,q] per head: PSUM init with bias via identity matmul, then
      += kT_h^T @ qT_h as K=32 row-tiled matmuls (4 heads share the PE array)
  p = exp(scoresT + maskterm)        (ACT, mask folds into per-partition bias)
  oT [hd,q] = v_h^T @ pT             (col-tiled 4 heads -> one [128,q] psum)
  denom     = ones^T @ pT            (col-tiled; 32x-replicated rows for free)
  oT_norm = oT * recip(denom)        (one DVE mult, alignment exact)
  out = oT_norm^T @ Wo + bo          (lhsT=oT slices)

All matmuls run as float32r (1 cycle/row for N>=256).
Self-contained: hardcodes all shapes; host side only reshapes/shards.
"""

import os
import sys

import numpy as np

if "/opt/trn_rl_repo" not in sys.path:
    sys.path.insert(0, "/opt/trn_rl_repo")

import concourse.bass as bass
import concourse.bacc as bacc
import concourse.tile as tile
from concourse import mybir
from concourse.bass_utils import run_bass_kernel_spmd

S, Q, KV, C, H, D = 64, 512, 512, 256, 8, 32
NCORES = 8
SLOC = S // NCORES          # 8 s-rows per core
HD = H * D                  # 256
OFF_WQ, OFF_WK, OFF_WV, OFF_WO = 0, 512, 1024, 1536
OFF_ID, OFF_BO, OFF_MC, OFF_ONE = 2048, 2176, 3200, 3232
OFF_ZERO = 3248
OFF_BIAS = 3312
BLOB_COLS = OFF_BIAS + H * 4 * Q    # 19648
F32 = mybir.dt.float32
F32R = mybir.dt.float32r
F16 = mybir.dt.float16
EXP = mybir.ActivationFunctionType.Exp

LAST_RESULT = None          # test.py reads exec_time/profile from here
_COMPILED = None


def build_nc(repeat=1, ebias_split=False, no_den=False, no_bias=False):
    from contextlib import ExitStack

    nc = bacc.Bacc("TRN2", target_bir_lowering=False, debug=False,
                   enable_asserts=False, num_devices=NCORES)
    blob = nc.declare_dram_parameter("blob", [128, BLOB_COLS], F32R, isOutput=False)
    xqT = nc.declare_dram_parameter("xqT", [SLOC, C, Q], F32R, isOutput=False)
    xkvT = nc.declare_dram_parameter("xkvT", [SLOC, C, KV], F32R, isOutput=False)
    out = nc.declare_dram_parameter("out", [SLOC, Q, C], F32, isOutput=True)

    with tile.TileContext(nc) as tc, ExitStack() as ctx:
        singles = ctx.enter_context(tc.tile_pool(name="singles", bufs=1))
        qT_pool = ctx.enter_context(tc.tile_pool(name="qTp", bufs=2))
        kT_pool = ctx.enter_context(tc.tile_pool(name="kTp", bufs=2))
        v_pool = ctx.enter_context(tc.tile_pool(name="vp", bufs=2))
        p_pool = ctx.enter_context(tc.tile_pool(name="pp", bufs=6))
        recip_pool = ctx.enter_context(tc.tile_pool(name="rp", bufs=1))
        oT_pool = ctx.enter_context(tc.tile_pool(name="oTp", bufs=2))
        out_pool = ctx.enter_context(tc.tile_pool(name="outp", bufs=2))

        # ---- constants: ONE blob DMA so every dependent matmul needs only
        # one DMA-queue wait (HW limit: 2 sync waits per instruction) ----
        blob_t = singles.tile([128, BLOB_COLS], F32R)
        nc.sync.dma_start(out=blob_t[:], in_=blob[:])
        wq_t = blob_t[:, OFF_WQ:OFF_WQ + 512].rearrange("p (c m) -> p c m", c=2)
        wk_t = blob_t[:, OFF_WK:OFF_WK + 512].rearrange("p (c m) -> p c m", c=2)
        wv_t = blob_t[:, OFF_WV:OFF_WV + 512].rearrange("p (c m) -> p c m", c=2)
        wo_t = blob_t[:, OFF_WO:OFF_WO + 512].rearrange("p (c m) -> p c m", c=2)
        ident_t = blob_t[:, OFF_ID:OFF_ID + 128]
        bo4_t = blob_t[:, OFF_BO:OFF_BO + 1024].bitcast(F32)
        maskc_t = blob_t[:, OFF_MC:OFF_MC + SLOC * 4].bitcast(F32)
        ones_t = singles.tile([128, 32], F16)
        nc.vector.memset(ones_t[:], 1.0)
        zeros_t = singles.tile([128, 128], F16)
        nc.vector.memset(zeros_t[:], 0.0)
        biasC = blob_t[:, OFF_BIAS:OFF_BIAS + H * 4 * Q].rearrange(
            "p (h b q) -> p h b q", h=H, b=4)
        # exp(bias) for heads 4..7, fp16: g=1 applies bias as a DVE multiply
        # after the exp instead of a PE PSUM-init matmul.
        ebias16 = singles.tile([128, 4, 4, Q], F16)
        for hh in range(4):
            nc.scalar.activation(
                out=ebias16[:, hh], in_=biasC[:, 4 + hh].bitcast(F32), func=EXP)
        # whole-core x tensors (16KB/partition each)
        xq_all = singles.tile([128, SLOC, 2, Q], F32R)
        nc.sync.dma_start(
            out=xq_all[:], in_=xqT[:].rearrange("s (c p) q -> p s c q", p=128))
        xkv_all = singles.tile([128, SLOC, 2, KV], F32R)
        nc.sync.dma_start(
            out=xkv_all[:], in_=xkvT[:].rearrange("s (c p) q -> p s c q", p=128))

        ps_sc = ctx.enter_context(
            tc.tile_pool(name="ps_sc", bufs=3, space="PSUM"))
        ps_att = ctx.enter_context(
            tc.tile_pool(name="ps_att", bufs=2, space="PSUM"))

        def emit_proj(s):
            """qT/kT/v projections for row s (pipelined one s ahead)."""
            xq_t = xq_all[:, s]
            xkv_t = xkv_all[:, s]
            ps_q = ps_sc.tile([128, 1024], F32, tag="ps")
            for hc in range(2):
                for cc in range(2):
                    nc.tensor.matmul(
                        ps_q[:, 512 * hc:512 * hc + 512],
                        wq_t[:, cc, 128 * hc:128 * hc + 128],
                        xq_t[:, cc, :],
                        start=(cc == 0), stop=(cc == 1))
            qT_t = qT_pool.tile([128, 2, Q], F32R, tag="qT")
            nc.vector.tensor_copy(
                qT_t[:], ps_q[:].rearrange("p (a q) -> p a q", a=2))

            ps_k = ps_sc.tile([128, 1024], F32, tag="ps")
            for hc in range(2):
                for cc in range(2):
                    nc.tensor.matmul(
                        ps_k[:, 512 * hc:512 * hc + 512],
                        wk_t[:, cc, 128 * hc:128 * hc + 128],
                        xkv_t[:, cc, :],
                        start=(cc == 0), stop=(cc == 1))
            kT_t = kT_pool.tile([128, 2, KV], F32R, tag="kT")
            nc.vector.tensor_copy(
                kT_t[:], ps_k[:].rearrange("p (a q) -> p a q", a=2))

            # v [kv, hd]: 4 kv-blocks of [128, 256], packed 2 per bank-pair
            ps_v = ps_sc.tile([128, 1024], F32, tag="ps")
            ps_v2 = ps_sc.tile([128, 1024], F32, tag="ps")
            for b in range(4):
                tgt = ps_v if b < 2 else ps_v2
                col = 256 * (b % 2)
                for cc in range(2):
                    nc.tensor.matmul(
                        tgt[:, col:col + 256],
                        xkv_t[:, cc, 128 * b:128 * b + 128],
                        wv_t[:, cc, :],
                        start=(cc == 0), stop=(cc == 1))
            v_t = v_pool.tile([128, 4, HD], F16, tag="v")
            nc.vector.tensor_copy(
                v_t[:, 0:2, :], ps_v[:, :512].rearrange("p (a m) -> p a m", a=2))
            nc.vector.tensor_copy(
                v_t[:, 2:4, :], ps_v2[:, :512].rearrange("p (a m) -> p a m", a=2))
            return qT_t, kT_t, v_t

        s_list = [s for _ in range(repeat) for s in range(SLOC)]
        proj_next = emit_proj(s_list[0])
        for si, s in enumerate(s_list):
            qT_t, kT_t, v_t = proj_next

            # ---- attention: head-group g outer so att/den use 1 bank each,
            # leaving ps_sc room for 3 double-buffered score slots ----
            oT_t = oT_pool.tile([128, 1024], F32R, tag="oT")
            for g in range(2):
                att_g = ps_att.tile([128, 512], F32, tag="att")
                den_g = ps_att.tile([128, 512], F32, tag="att")
                for b in range(4):
                    sc_a = ps_sc.tile([128, 1024], F32, tag="ps")
                    sc_b = ps_sc.tile([128, 1024], F32, tag="ps")
                    scs = (sc_a, sc_b)
                    if (g == 0 or not ebias_split) and not no_bias:
                        for j in range(4):
                            h = 4 * g + j  # noqa
                            nc.tensor.matmul(
                                scs[j // 2][:, 512 * (j % 2):512 * (j % 2) + 512],
                                ident_t[:],
                                biasC[:, h, b, :],
                                start=True, stop=False, skip_group_check=True)
                    for j in range(4):      # 4-way row-packed K=32 score MMs
                        nc.tensor.matmul(
                            scs[j // 2][:, 512 * (j % 2):512 * (j % 2) + 512],
                            kT_t[32 * j:32 * j + 32, g, 128 * b:128 * b + 128],
                            qT_t[32 * j:32 * j + 32, g, :],
                            start=(g == 1 and ebias_split) or no_bias, stop=True, skip_group_check=True,
                            tile_position=(32 * j, 0))
                    p_a = p_pool.tile([128, 1024], F16, tag="p")
                    nc.scalar.activation(
                        out=p_a[:], in_=sc_a[:], func=EXP,
                        bias=maskc_t[:, s * 4 + b:s * 4 + b + 1], scale=1.0)
                    p_b = p_pool.tile([128, 1024], F16, tag="p")
                    nc.scalar.activation(
                        out=p_b[:], in_=sc_b[:], func=EXP,
                        bias=maskc_t[:, s * 4 + b:s * 4 + b + 1], scale=1.0)
                    if g == 1 and ebias_split:  # fold exp(bias) in on the DVE
                        pm_a = p_pool.tile([128, 1024], F16, tag="p")
                        nc.vector.tensor_mul(
                            pm_a[:].rearrange("p (a q) -> p a q", a=2),
                            p_a[:].rearrange("p (a q) -> p a q", a=2),
                            ebias16[:, 0:2, b, :])
                        pm_b = p_pool.tile([128, 1024], F16, tag="p")
                        nc.vector.tensor_mul(
                            pm_b[:].rearrange("p (a q) -> p a q", a=2),
                            p_b[:].rearrange("p (a q) -> p a q", a=2),
                            ebias16[:, 2:4, b, :])
                        p_a, p_b = pm_a, pm_b

                    # col-tiled AV + denom.  start=True clears has_written for
                    # the WHOLE bank, so each bank opens with one full-width
                    # zeros matmul; col-tiles then accumulate via start=False.
                    if b == 0:
                        nc.tensor.matmul(
                            att_g[:], zeros_t[:], p_a[:, :512],
                            start=True, stop=False, skip_group_check=True)
                        nc.tensor.matmul(
                            den_g[:], zeros_t[:], p_a[:, :512],
                            start=True, stop=False, skip_group_check=True)
                    for j in range(4):
                        pt = (p_a, p_a, p_b, p_b)[j]
                        rhs = pt[:, 512 * (j % 2):512 * (j % 2) + 512]
                        nc.tensor.matmul(
                            att_g[32 * j:32 * j + 32, :],
                            v_t[:, b, 32 * (4 * g + j):32 * (4 * g + j) + 32],
                            rhs,
                            start=False, stop=(b == 3 and j == 3),
                            skip_group_check=True, tile_position=(0, 32 * j))
                    for j in range(4 * (not no_den)):
                        pt = (p_a, p_a, p_b, p_b)[j]
                        rhs = pt[:, 512 * (j % 2):512 * (j % 2) + 512]
                        nc.tensor.matmul(
                            den_g[32 * j:32 * j + 32, :],
                            ones_t[:],
                            rhs,
                            start=False, stop=(b == 3 and j == 3),
                            skip_group_check=True, tile_position=(0, 32 * j))

                # ---- normalize this head-group ----
                recip_t = recip_pool.tile([128, 512], F32, tag="recip")
                nc.vector.reciprocal_approx_fast(out=recip_t[:], in_=den_g[:])
                nc.vector.tensor_mul(oT_t[:, 512 * g:512 * g + 512],
                                     att_g[:], recip_t[:])
                if g == 0 and si + 1 < len(s_list):
                    proj_next = emit_proj(s_list[si + 1])

            # ---- output projection (one ps_sc slot: 2 qblocks per bank) ----
            po = ps_sc.tile([128, 1024], F32, tag="ps")
            for qb in range(4):
                for c in range(2):
                    nc.tensor.matmul(
                        po[:, 256 * qb:256 * qb + 256],
                        oT_t[:, 512 * c + 128 * qb:512 * c + 128 * qb + 128],
                        wo_t[:, c, :],
                        start=(c == 0), stop=(c == 1))
            out_t = out_pool.tile([128, 4 * C], F32, tag="out")
            nc.vector.tensor_add(out_t[:], po[:], bo4_t[:])
            nc.gpsimd.dma_start(
                out=out[s].rearrange("(b p) c -> p b c", p=128),
                in_=out_t[:].rearrange("p (b c) -> p b c", b=4))

    nc.compile()
    return nc


def _get_compiled():
    global _COMPILED
    if _COMPILED is None:
        _COMPILED = build_nc()
    return _COMPILED


def prep_in_maps(input_q, input_kv, mask, bias, Wq, Wkv, Wo, bo):
    input_q = np.asarray(input_q, dtype=np.float32)
    input_kv = np.asarray(input_kv, dtype=np.float32)
    mask = np.asarray(mask, dtype=np.float32)
    bias = np.asarray(bias, dtype=np.float32)
    Wq = np.asarray(Wq, dtype=np.float32)
    Wkv = np.asarray(Wkv, dtype=np.float32)
    Wo = np.asarray(Wo, dtype=np.float32)
    bo = np.asarray(bo, dtype=np.float32)

    # [h, kv, q] bias, then packed as [p, h, b, q]
    biasT = np.transpose(bias[0, 0], (0, 2, 1))
    bias_pk = np.ascontiguousarray(
        biasT.reshape(H, 4, 128, Q).transpose(2, 0, 1, 3).reshape(128, H * 4 * Q))

    def chunks2(w):  # [C, M] -> [p, (c m)] with 128-row C-chunks
        return w.reshape(2, 128, w.shape[1]).transpose(1, 0, 2).reshape(128, -1)

    wq_s = chunks2(Wq / np.sqrt(np.float32(D)))
    wk_pk = chunks2(Wkv[:, :HD])
    wv_pk = chunks2(Wkv[:, HD:])
    wo_pk = chunks2(Wo)
    bo4 = np.tile(bo[None, :], (128, 4))
    ident = np.eye(128, dtype=np.float32)
    ones16 = np.ones((128, 32), np.float16).view(np.float32)  # 16 f32 words

    in_maps = []
    for cid in range(NCORES):
        sl = slice(cid * SLOC, (cid + 1) * SLOC)
        xqT = np.ascontiguousarray(np.transpose(input_q[0, sl], (0, 2, 1)))
        xkvT = np.ascontiguousarray(np.transpose(input_kv[0, sl], (0, 2, 1)))
        m = mask[0, sl, 0, 0, :]                       # [SLOC, KV]
        term = (m - 1.0) * np.float32(1.0e9)
        maskcol = term.reshape(SLOC, 4, 128).transpose(2, 0, 1).reshape(128, SLOC * 4)
        blob = np.zeros((128, BLOB_COLS), np.float32)
        blob[:, OFF_WQ:OFF_WQ + 512] = wq_s
        blob[:, OFF_WK:OFF_WK + 512] = wk_pk
        blob[:, OFF_WV:OFF_WV + 512] = wv_pk
        blob[:, OFF_WO:OFF_WO + 512] = wo_pk
        blob[:, OFF_ID:OFF_ID + 128] = ident
        blob[:, OFF_BO:OFF_BO + 1024] = bo4
        blob[:, OFF_MC:OFF_MC + SLOC * 4] = maskcol
        blob[:, OFF_ONE:OFF_ONE + 16] = ones16
        blob[:, OFF_BIAS:] = bias_pk
        in_maps.append(dict(blob=blob, xqT=xqT, xkvT=xkvT))

    return in_maps


def kernel(input_q, input_kv, mask, bias, Wq, Wkv, Wo, bo):
    global LAST_RESULT
    nc = _get_compiled()
    in_maps = prep_in_maps(input_q, input_kv, mask, bias, Wq, Wkv, Wo, bo)
    trace = bool(int(os.environ.get("KERNEL_TRACE", "0")))
    LAST_RESULT = run_bass_kernel_spmd(
        nc, in_maps, list(range(NCORES)), trace=trace)
    outs = [LAST_RESULT.results[cid]["out"] for cid in range(NCORES)]
    full = np.concatenate(outs, axis=0)[None]          # [1, S, Q, C]
    return np.ascontiguousarray(full.astype(np.float32))


if __name__ == "__main__":
    rng = np.random.default_rng(0)
    demo = dict(
        input_q=rng.standard_normal((1, S, Q, C), dtype=np.float32),
        input_kv=rng.standard_normal((1, S, KV, C), dtype=np.float32),
        mask=np.ones((1, S, 1, 1, KV), np.float32),
        bias=rng.standard_normal((1, 1, H, Q, KV), dtype=np.float32) * 0.1,
        Wq=rng.standard_normal((C, HD), dtype=np.float32) * 0.06,
        Wkv=rng.standard_normal((C, 2 * HD), dtype=np.float32) * 0.05,
        Wo=rng.standard_normal((HD, C), dtype=np.float32) * 0.02,
        bo=np.zeros((C,), np.float32),
    )
    o = kernel(**demo)
    print("out", o.shape, o.dtype, float(np.abs(o).max()))



# revision 9
# speedup vs baseline: 897.8984x; 897.8984x over previous
"""CrossAttentionNoGate Trainium2 kernel (v2, f16 pipeline).

Shards the MSA-row dim S (=64) across 8 NeuronCores (8 rows/core, fully
data-parallel, no collectives). All matmul operands are fp16 (psum stays
f32): fp32r moving operands block PE tile-concurrency, f16 unlocks it and
halves DMA. Per core, per s-row, work is chunked as (g, b, jp):
g = head-group (4 heads), b = kv-block (128), jp = head-pair within g.

  phase P (interleaved):  qT/kT/v projections for row s+1
  scores chunk [128,1024] = 2 heads x 512 q, kv-block b on partitions:
     g < EB_PE/4 : PSUM-init with bias via f16 identity matmul, qk on top
     g >= EB_PE/4: qk straight (start=True), exp(bias) folded in on DVE
  (default eb_pe=0: all bias via DVE exp(bias) multiply — best measured)
  p = exp(scores + maskterm)   one ACT per chunk ([128,1024], f16 out)
  att|den accumulate into one [128,1024] slot (att cols 0-511, den 512+),
     col-tiled per head, zeros-matmul opens each bank's has_written
  oT = att * recip(den); out = oT^T @ Wo + bo -> f16 DMA (host casts f32)

PSUM budget: 2 score slots (4 banks) + 2 att + 2 den slots = 8 banks.
Self-contained: hardcodes all shapes; host side only reshapes/shards.
"""

import os
import sys

import numpy as np

if "/opt/trn_rl_repo" not in sys.path:
    sys.path.insert(0, "/opt/trn_rl_repo")

import concourse.bass as bass
import concourse.bacc as bacc
import concourse.tile as tile
from concourse import mybir
from concourse.bass_utils import run_bass_kernel_spmd

S, Q, KV, C, H, D = 64, 512, 512, 256, 8, 32
NCORES = 8
SLOC = S // NCORES          # 8 s-rows per core
HD = H * D                  # 256
F32 = mybir.dt.float32
F16 = mybir.dt.float16
EXP = mybir.ActivationFunctionType.Exp

# f16 blob column offsets
OFF_WQ, OFF_WK, OFF_WV, OFF_WO = 0, 512, 1024, 1536
OFF_ID = 2048               # 128 cols identity f16
OFF_ONE = 2176              # 32 cols ones f16
OFF_BO = 2208               # 2048 f16 cols = 1024 f32 (bo tiled 4x)
OFF_MC = 4256               # 64 f16 cols = 32 f32 (mask per-partition terms)
OFF_BIAS = 4320             # H*4*Q = 16384 cols bias f16
BLOB_COLS = OFF_BIAS + H * 4 * Q    # 20704

LAST_RESULT = None          # test.py reads exec_time/profile from here
_COMPILED = None


def build_nc(repeat=1, eb_pe=0):
    """eb_pe: number of heads whose bias is PSUM-initialized on the PE
    (multiple of 4); remaining heads get exp(bias) multiplied on the DVE."""
    from contextlib import ExitStack

    n_pe_g = eb_pe // 4

    nc = bacc.Bacc("TRN2", target_bir_lowering=False, debug=False,
                   enable_asserts=False, num_devices=NCORES)
    blob = nc.declare_dram_parameter("blob", [128, BLOB_COLS], F16, isOutput=False)
    xqT = nc.declare_dram_parameter("xqT", [SLOC, C, Q], F16, isOutput=False)
    xkvT = nc.declare_dram_parameter("xkvT", [SLOC, C, KV], F16, isOutput=False)
    out = nc.declare_dram_parameter("out", [SLOC, Q, C], F16, isOutput=True)

    with tile.TileContext(nc) as tc, ExitStack() as ctx:
        singles = ctx.enter_context(tc.tile_pool(name="singles", bufs=1))
        qT_pool = ctx.enter_context(tc.tile_pool(name="qTp", bufs=2))
        kT_pool = ctx.enter_context(tc.tile_pool(name="kTp", bufs=2))
        v_pool = ctx.enter_context(tc.tile_pool(name="vp", bufs=2))
        p_pool = ctx.enter_context(tc.tile_pool(name="pp", bufs=6))
        recip_pool = ctx.enter_context(tc.tile_pool(name="rp", bufs=2))
        oT_pool = ctx.enter_context(tc.tile_pool(name="oTp", bufs=2))
        out_pool = ctx.enter_context(tc.tile_pool(name="outp", bufs=2))

        # ---- constants ----
        blob_t = singles.tile([128, BLOB_COLS], F16)
        nc.sync.dma_start(out=blob_t[:], in_=blob[:])
        wq_t = blob_t[:, OFF_WQ:OFF_WQ + 512].rearrange("p (c m) -> p c m", c=2)
        wk_t = blob_t[:, OFF_WK:OFF_WK + 512].rearrange("p (c m) -> p c m", c=2)
        wv_t = blob_t[:, OFF_WV:OFF_WV + 512].rearrange("p (c m) -> p c m", c=2)
        wo_t = blob_t[:, OFF_WO:OFF_WO + 512].rearrange("p (c m) -> p c m", c=2)
        ident_t = blob_t[:, OFF_ID:OFF_ID + 128]
        ones_t = blob_t[:, OFF_ONE:OFF_ONE + 32]
        bo4_t = blob_t[:, OFF_BO:OFF_BO + 2048].bitcast(F32)
        maskc_t = blob_t[:, OFF_MC:OFF_MC + 64].bitcast(F32)
        biasC = blob_t[:, OFF_BIAS:OFF_BIAS + H * 4 * Q].rearrange(
            "p (h b q) -> p h b q", h=H, b=4)
        zeros_t = singles.tile([128, 128], F16)
        nc.vector.memset(zeros_t[:], 0.0)

        # exp(bias) f16 for DVE-bias head groups
        n_dve_g = 2 - n_pe_g
        if n_dve_g:
            ebias16 = singles.tile([128, n_dve_g * 4, 4, Q], F16)
            for gi in range(n_dve_g):
                for jj in range(4):
                    h = (n_pe_g + gi) * 4 + jj
                    for b in range(4):
                        nc.scalar.activation(
                            out=ebias16[:, gi * 4 + jj, b], in_=biasC[:, h, b],
                            func=EXP)

        # whole-core x tensors, DMA'd per row so row 0 starts early
        xq_all = singles.tile([128, SLOC, 2, Q], F16)
        xkv_all = singles.tile([128, SLOC, 2, KV], F16)
        for s in range(SLOC):
            nc.sync.dma_start(
                out=xq_all[:, s],
                in_=xqT[s].rearrange("(c p) q -> p c q", p=128))
            nc.sync.dma_start(
                out=xkv_all[:, s],
                in_=xkvT[s].rearrange("(c p) q -> p c q", p=128))

        ps_sc = ctx.enter_context(
            tc.tile_pool(name="ps_sc", bufs=2, space="PSUM"))
        ps_at = ctx.enter_context(
            tc.tile_pool(name="ps_at", bufs=2, space="PSUM"))
        ps_dn = ctx.enter_context(
            tc.tile_pool(name="ps_dn", bufs=2, space="PSUM"))

        def emit_proj_q(s):
            ps_q = ps_sc.tile([128, 1024], F32, tag="ps")
            for hc in range(2):
                for cc in range(2):
                    nc.tensor.matmul(
                        ps_q[:, 512 * hc:512 * hc + 512],
                        wq_t[:, cc, 128 * hc:128 * hc + 128],
                        xq_all[:, s, cc, :],
                        start=(cc == 0), stop=(cc == 1))
            qT_t = qT_pool.tile([128, 2, Q], F16, tag="qT")
            nc.vector.tensor_copy(
                qT_t[:], ps_q[:].rearrange("p (a q) -> p a q", a=2))
            return qT_t

        def emit_proj_k(s):
            ps_k = ps_sc.tile([128, 1024], F32, tag="ps")
            for hc in range(2):
                for cc in range(2):
                    nc.tensor.matmul(
                        ps_k[:, 512 * hc:512 * hc + 512],
                        wk_t[:, cc, 128 * hc:128 * hc + 128],
                        xkv_all[:, s, cc, :],
                        start=(cc == 0), stop=(cc == 1))
            kT_t = kT_pool.tile([128, 2, KV], F16, tag="kT")
            nc.vector.tensor_copy(
                kT_t[:], ps_k[:].rearrange("p (a q) -> p a q", a=2))
            return kT_t

        def emit_proj_v(s):
            # v [kv, hd]: 4 kv-blocks of [128, 256] packed in one psum slot
            ps_v = ps_sc.tile([128, 1024], F32, tag="ps")
            v_t = v_pool.tile([128, 4, HD], F16, tag="v")
            for b in range(4):
                for cc in range(2):
                    nc.tensor.matmul(
                        ps_v[:, 256 * b:256 * b + 256],
                        xkv_all[:, s, cc, 128 * b:128 * b + 128],
                        wv_t[:, cc, :],
                        start=(cc == 0), stop=(cc == 1))
            nc.vector.tensor_copy(
                v_t[:], ps_v[:].rearrange("p (a m) -> p a m", a=4))
            return v_t

        projs = {0: [emit_proj_q(0), emit_proj_k(0), emit_proj_v(0)]}

        s_list = [s for _ in range(repeat) for s in range(SLOC)]
        for si, s in enumerate(s_list):
            qT_t, kT_t, v_t = projs.pop(si)
            oT_t = oT_pool.tile([128, 2, Q], F16, tag="oT")
            for g in range(2):
                att_t = ps_at.tile([128, 512], F32, tag="at")
                den_t = ps_dn.tile([128, 512], F32, tag="dn")
                # open both banks' has_written with one zeros matmul each
                nc.tensor.matmul(att_t[:], zeros_t[:], kT_t[:, 0, 0:512],
                                 start=True, stop=False, skip_group_check=True)
                nc.tensor.matmul(den_t[:], zeros_t[:], kT_t[:, 0, 0:512],
                                 start=True, stop=False, skip_group_check=True)
                pe_bias = g < n_pe_g
                for b in range(4):
                    for jp in range(2):
                        sc = ps_sc.tile([128, 1024], F32, tag="ps")
                        if pe_bias:
                            for i in range(2):
                                jj = 2 * jp + i
                                nc.tensor.matmul(
                                    sc[:, 512 * i:512 * i + 512],
                                    ident_t[:],
                                    biasC[:, 4 * g + jj, b, :],
                                    start=True, stop=False,
                                    skip_group_check=True)
                        for i in range(2):      # qk, row-tiled per head
                            jj = 2 * jp + i
                            nc.tensor.matmul(
                                sc[:, 512 * i:512 * i + 512],
                                kT_t[32 * jj:32 * jj + 32, g,
                                     128 * b:128 * b + 128],
                                qT_t[32 * jj:32 * jj + 32, g, :],
                                start=not pe_bias, stop=True,
                                skip_group_check=True,
                                tile_position=(32 * jj, 0))
                        p_t = p_pool.tile([128, 1024], F16, tag="p")
                        nc.scalar.activation(
                            out=p_t[:], in_=sc[:], func=EXP,
                            bias=maskc_t[:, s * 4 + b:s * 4 + b + 1], scale=1.0)
                        if not pe_bias:   # fold exp(bias) in on the DVE
                            pm = p_pool.tile([128, 1024], F16, tag="p")
                            nc.vector.tensor_mul(
                                pm[:].rearrange("p (a q) -> p a q", a=2),
                                p_t[:].rearrange("p (a q) -> p a q", a=2),
                                ebias16[:, (g - n_pe_g) * 4 + 2 * jp:
                                        (g - n_pe_g) * 4 + 2 * jp + 2, b, :])
                            p_t = pm
                        for i in range(2):      # AV + denom, col-tiled
                            jj = 2 * jp + i
                            rhs = p_t[:, 512 * i:512 * i + 512]
                            nc.tensor.matmul(
                                att_t[32 * jj:32 * jj + 32, :],
                                v_t[:, b, 32 * (4 * g + jj):32 * (4 * g + jj) + 32],
                                rhs,
                                start=False, stop=(b == 3 and jp == 1),
                                skip_group_check=True,
                                tile_position=(0, 32 * jj))
                            nc.tensor.matmul(
                                den_t[32 * jj:32 * jj + 32, :],
                                ones_t[:],
                                rhs,
                                start=False, stop=(b == 3 and jp == 1),
                                skip_group_check=True,
                                tile_position=(0, 32 * jj))
                    # interleave next row's projections mid-stream
                    if g == 0 and b >= 1 and si + 1 < len(s_list):
                        sn = s_list[si + 1]
                        if b == 1:
                            projs[si + 1] = [emit_proj_q(sn), None, None]
                        elif b == 2:
                            projs[si + 1][1] = emit_proj_k(sn)
                        elif b == 3:
                            projs[si + 1][2] = emit_proj_v(sn)

                # ---- normalize this head-group ----
                recip_t = recip_pool.tile([128, 512], F32, tag="recip")
                nc.vector.reciprocal_approx_fast(
                    out=recip_t[:], in_=den_t[:])
                nc.vector.tensor_mul(oT_t[:, g, :], att_t[:], recip_t[:])

            # ---- output projection ----
            po = ps_sc.tile([128, 1024], F32, tag="ps")
            for qb in range(4):
                for cchunk in range(2):
                    nc.tensor.matmul(
                        po[:, 256 * qb:256 * qb + 256],
                        oT_t[:, cchunk, 128 * qb:128 * qb + 128],
                        wo_t[:, cchunk, :],
                        start=(cchunk == 0), stop=(cchunk == 1))
            out_t = out_pool.tile([128, 4 * C], F16, tag="out")
            nc.vector.tensor_add(out_t[:], po[:], bo4_t[:])
            nc.gpsimd.dma_start(
                out=out[s].rearrange("(b p) c -> p b c", p=128),
                in_=out_t[:].rearrange("p (b c) -> p b c", b=4))

    nc.compile()
    return nc


def _get_compiled():
    global _COMPILED
    if _COMPILED is None:
        _COMPILED = build_nc()
    return _COMPILED


def prep_in_maps(input_q, input_kv, mask, bias, Wq, Wkv, Wo, bo):
    input_q = np.asarray(input_q, dtype=np.float32)
    input_kv = np.asarray(input_kv, dtype=np.float32)
    mask = np.asarray(mask, dtype=np.float32)
    bias = np.asarray(bias, dtype=np.float32)
    Wq = np.asarray(Wq, dtype=np.float32)
    Wkv = np.asarray(Wkv, dtype=np.float32)
    Wo = np.asarray(Wo, dtype=np.float32)
    bo = np.asarray(bo, dtype=np.float32)

    # [h, kv, q] bias, then packed as [p, h, b, q] fp16
    biasT = np.transpose(bias[0, 0], (0, 2, 1))
    bias_pk = np.ascontiguousarray(
        biasT.reshape(H, 4, 128, Q).transpose(2, 0, 1, 3)
        .reshape(128, H * 4 * Q)).astype(np.float16)

    def chunks2(w):  # [C, M] -> [p, (c m)] with 128-row C-chunks
        return (w.reshape(2, 128, w.shape[1]).transpose(1, 0, 2)
                .reshape(128, -1).astype(np.float16))

    wq_s = chunks2(Wq / np.sqrt(np.float32(D)))
    wk_pk = chunks2(Wkv[:, :HD])
    wv_pk = chunks2(Wkv[:, HD:])
    wo_pk = chunks2(Wo)
    bo4 = np.ascontiguousarray(
        np.tile(bo[None, :], (128, 4)).astype(np.float32)).view(np.float16)
    ident = np.eye(128, dtype=np.float16)
    ones16 = np.ones((128, 32), np.float16)

    in_maps = []
    for cid in range(NCORES):
        sl = slice(cid * SLOC, (cid + 1) * SLOC)
        xqT = np.ascontiguousarray(
            np.transpose(input_q[0, sl], (0, 2, 1)).astype(np.float16))
        xkvT = np.ascontiguousarray(
            np.transpose(input_kv[0, sl], (0, 2, 1)).astype(np.float16))
        m = mask[0, sl, 0, 0, :]                       # [SLOC, KV]
        term = (m - 1.0) * np.float32(1.0e9)
        maskcol = np.ascontiguousarray(
            term.reshape(SLOC, 4, 128).transpose(2, 0, 1)
            .reshape(128, SLOC * 4).astype(np.float32)).view(np.float16)
        blob = np.zeros((128, BLOB_COLS), np.float16)
        blob[:, OFF_WQ:OFF_WQ + 512] = wq_s
        blob[:, OFF_WK:OFF_WK + 512] = wk_pk
        blob[:, OFF_WV:OFF_WV + 512] = wv_pk
        blob[:, OFF_WO:OFF_WO + 512] = wo_pk
        blob[:, OFF_ID:OFF_ID + 128] = ident
        blob[:, OFF_ONE:OFF_ONE + 32] = ones16
        blob[:, OFF_BO:OFF_BO + 2048] = bo4
        blob[:, OFF_MC:OFF_MC + 64] = maskcol
        blob[:, OFF_BIAS:] = bias_pk
        in_maps.append(dict(blob=blob, xqT=xqT, xkvT=xkvT))

    return in_maps


def kernel(input_q, input_kv, mask, bias, Wq, Wkv, Wo, bo):
    global LAST_RESULT
    nc = _get_compiled()
    in_maps = prep_in_maps(input_q, input_kv, mask, bias, Wq, Wkv, Wo, bo)
    trace = bool(int(os.environ.get("KERNEL_TRACE", "0")))
    LAST_RESULT = run_bass_kernel_spmd(
        nc, in_maps, list(range(NCORES)), trace=trace)
    outs = [LAST_RESULT.results[cid]["out"] for cid in range(NCORES)]
    full = np.concatenate(outs, axis=0)[None]          # [1, S, Q, C] f16
    return np.ascontiguousarray(full.astype(np.float32))


if __name__ == "__main__":
    rng = np.random.default_rng(0)
    demo = dict(
        input_q=rng.standard_normal((1, S, Q, C), dtype=np.float32),
        input_kv=rng.standard_normal((1, S, KV, C), dtype=np.float32),
        mask=np.ones((1, S, 1, 1, KV), np.float32),
        bias=rng.standard_normal((1, 1, H, Q, KV), dtype=np.float32) * 0.1,
        Wq=rng.standard_normal((C, HD), dtype=np.float32) * 0.06,
        Wkv=rng.standard_normal((C, 2 * HD), dtype=np.float32) * 0.05,
        Wo=rng.standard_normal((HD, C), dtype=np.float32) * 0.02,
        bo=np.zeros((C,), np.float32),
    )
    o = kernel(**demo)
    print("out", o.shape, o.dtype, float(np.abs(o).max()))
